# revision 1
# baseline (speedup 1.0000x reference)
"""Causal multi-head attention (B=2, S=2048, D=1024, H=16) on 8 TRN2 NeuronCores.

Sharding: sequence-parallel. Cores 0-3 handle batch 0, cores 4-7 batch 1.
Within a batch group, the core with local index l owns the mirrored pair of
256-row chunks (A = rows [256l, 256l+256), B = rows [256(7-l), 256(8-l))),
which equalizes causal attention work across cores. A uniform 24-job
structure per head serves all cores from one SPMD program; per-core
host-computed masks select valid/diagonal/invalid kv blocks.

v2 changes vs baseline:
  - Phase 2 interleaves k/v computation in 2-pair groups so each pair's
    kT+v AllGather launches as early as possible (CC engine is the
    long pole; its 8 transfers must pipeline behind attention).
  - Attention processes jobs in pairs: 4 QK matmuls accumulate into one
    [128,1024] PSUM stripe, ONE exp activation (amortizes ACT's ~352-cycle
    fixed overhead) and ONE mask multiply per job-pair.
  - PSUM ctx tiles initialize via start=True on the first PV matmul
    (no zero-matmul preamble).
  - Softmax denominators collect into a [16,512] SBUF tile; one deferred
    DVE reciprocal + per-pair PE broadcast matmul (rows 2p/2p+1 of the
    reciprocal map to partitions 0:64/64:128) replaces the per-pair
    single-partition reciprocal + DMA round-trip broadcast.
"""

import numpy as np

B, S, D = 2, 2048, 1024
H = 16
HD = 64
NCORES = 8
CHUNK = 256          # rows per chunk; 2 chunks per core
SLOC = 2 * CHUNK     # rows per core
NPAIR = H // 2       # head pairs
NJOB = 24            # uniform job count per head: 16 B-phase + 8 A-phase
NJP = NJOB // 2      # job pairs
KT_P = 128 * SLOC    # kT elems per pair block
V_P = SLOC * 130     # v(+ones) elems per pair block
PAIRSZ = KT_P + V_P

_CACHE = {}


def _build_nc():
    import ml_dtypes
    import concourse.bass as bass
    import concourse.bacc as bacc
    import concourse.mybir as mybir
    import concourse.tile as tile

    f32 = mybir.dt.float32
    bf16 = mybir.dt.bfloat16
    MULT = mybir.AluOpType.mult
    ADD = mybir.AluOpType.add
    EXP = mybir.ActivationFunctionType.Exp

    nc = bacc.Bacc(num_devices=NCORES)

    x_in = nc.dram_tensor("x_local", [SLOC, D], bf16, kind="ExternalInput")
    wqkp_in = nc.dram_tensor("w_qk_p", [128, 16, 8, 128], bf16, kind="ExternalInput")
    wv_in = nc.dram_tensor("w_v", [D, D], bf16, kind="ExternalInput")
    bqk_in = nc.dram_tensor("b_qk_t", [128, 16], f32, kind="ExternalInput")
    bv_in = nc.dram_tensor("b_v_bc", [128, D], f32, kind="ExternalInput")
    wout_in = nc.dram_tensor("w_out", [D, D], bf16, kind="ExternalInput")
    masks_in = nc.dram_tensor("masks2", [128, NJP, 4 * CHUNK], bf16, kind="ExternalInput")
    y_out = nc.dram_tensor("y", [SLOC, D], f32, kind="ExternalOutput")

    cc_in = nc.dram_tensor("cc_in", [NPAIR * PAIRSZ], bf16)
    den_dram = nc.dram_tensor("den_scratch", [16, SLOC], f32)
    cc_outs = [nc.dram_tensor(f"cc_out_{p}", [4, PAIRSZ], bf16) for p in range(NPAIR)]
    groups = [[0, 1, 2, 3], [4, 5, 6, 7]]

    ident_h = nc.inline_tensor(np.eye(128).astype(ml_dtypes.bfloat16), name="ident_c")
    zeros_h = nc.inline_tensor(np.zeros((1, 512), ml_dtypes.bfloat16), name="zeros_c")
    # E[2p, p*128+c]=1 for c<64; E[2p+1, p*128+c]=1 for 64<=c<128: broadcast
    # matrix mapping denominator-reciprocal rows to ctxT partition halves.
    E_np = np.zeros((16, NPAIR * 128), np.float32)
    for p in range(NPAIR):
        E_np[2 * p, p * 128:p * 128 + 64] = 1.0
        E_np[2 * p + 1, p * 128 + 64:p * 128 + 128] = 1.0
    E_h = nc.inline_tensor(E_np.astype(ml_dtypes.bfloat16), name="e_bcast")

    def kt_view(p, r):   # [128 c, SLOC s] of rank r's pair block
        return cc_outs[p][r, 0:KT_P].rearrange("(c s) -> c s", s=SLOC)

    def v_view(p, r):    # [SLOC s, 130] of rank r's pair block
        return cc_outs[p][r, KT_P:].rearrange("(s c) -> s c", c=130)

    def chunk_owner(c):  # chunk -> (owner rank, row offset in that rank's 512)
        return (c, 0) if c < 4 else (7 - c, CHUNK)

    with tile.TileContext(nc) as tc:
        with tc.tile_pool(name="const", bufs=1) as cpool:
            it = cpool.tile([128, 128], bf16)
            nc.sync.dma_start(out=it[:], in_=ident_h[:])
            zt = cpool.tile([1, 512], bf16)
            nc.sync.dma_start(out=zt[:], in_=zeros_h[:])
            et = cpool.tile([16, NPAIR * 128], bf16)
            nc.sync.dma_start(out=et[:], in_=E_h[:])
            bqk = cpool.tile([128, 16], f32)
            nc.sync.dma_start(out=bqk[:], in_=bqk_in[:])
            bv = cpool.tile([128, D], f32)
            nc.sync.dma_start(out=bv[:], in_=bv_in[:])
            masks = cpool.tile([128, NJP, 4 * CHUNK], bf16)
            nc.sync.dma_start(out=masks[:], in_=masks_in[:])
            xT = cpool.tile([128, 8, SLOC], bf16)
            qT = cpool.tile([128, 8, SLOC], bf16)
            ctxT = cpool.tile([128, 8, SLOC], bf16)
            # denominator rows collect on partition 64 (same base as their
            # psc source row — no cross-partition lane shift); a DMA later
            # unflattens them to [16, 512] for one batched reciprocal.
            den_all = cpool.tile([65, NPAIR, 2, SLOC], f32)

            # ---- Phase 1: load x, transpose to xT ----
            with tc.tile_pool(name="ph1", bufs=3) as pool, \
                 tc.tile_pool(name="ph1p", bufs=4, space="PSUM") as psp:
                for sb in range(4):
                    xl = pool.tile([128, D], bf16, tag="xl")
                    nc.sync.dma_start(out=xl[:], in_=x_in[sb * 128:(sb + 1) * 128, :])
                    for db in range(8):
                        pst = psp.tile([128, 128], bf16, tag="tr")
                        nc.tensor.transpose(pst[:], xl[:, db * 128:(db + 1) * 128], it[:])
                        nc.vector.tensor_copy(out=xT[:, db, sb * 128:(sb + 1) * 128], in_=pst[:])

            # ---- Phase 2: QKV projection + per-pair AllGathers (earliest-first) ----
            with tc.tile_pool(name="ph2w", bufs=3) as wpool, \
                 tc.tile_pool(name="ph2wv", bufs=8) as wvpool, \
                 tc.tile_pool(name="ph2", bufs=3) as pool, \
                 tc.tile_pool(name="ph2p", bufs=2, space="PSUM") as psp:

                def qk_block(cb, out_sb):
                    wp = wpool.tile([128, 8, 128], bf16, tag="wp", name=f"wp_{cb}")
                    nc.sync.dma_start(out=wp[:], in_=wqkp_in[:, cb, :, :])
                    ps = psp.tile([128, SLOC], f32, tag="qk", name=f"psqk_{cb}")
                    for db in range(8):
                        nc.tensor.matmul(ps[:], wp[:, db, :], xT[:, db, :],
                                         start=(db == 0), stop=(db == 7))
                    nc.vector.tensor_scalar_add(out_sb, ps[:], bqk[:, cb:cb + 1])

                # 2-pair groups: kT then v then the pair AllGathers, so the CC
                # engine starts moving data while later pairs still compute.
                for g in range(4):
                    for j in range(2):
                        p = 2 * g + j
                        kt = pool.tile([128, SLOC], bf16, tag="kt", name=f"kt_{p}")
                        qk_block(8 + p, kt[:])
                        nc.sync.dma_start(
                            out=cc_in[p * PAIRSZ: p * PAIRSZ + KT_P]
                                .rearrange("(c s) -> c s", s=SLOC),
                            in_=kt[:])
                    wv = []
                    for db in range(8):
                        w = wvpool.tile([128, 256], bf16, tag="wv", name=f"wv_{g}_{db}")
                        nc.sync.dma_start(
                            out=w[:],
                            in_=wv_in[db * 128:(db + 1) * 128, g * 256:(g + 1) * 256])
                        wv.append(w)
                    for sb in range(4):
                        ps = psp.tile([128, 256], f32, tag="psv", name=f"psv_{g}_{sb}")
                        for db in range(8):
                            nc.tensor.matmul(ps[:], xT[:, db, sb * 128:(sb + 1) * 128],
                                             wv[db][:], start=(db == 0), stop=(db == 7))
                        vt = pool.tile([128, 2, 130], bf16, tag="vt", name=f"vt_{g}_{sb}")
                        nc.vector.tensor_tensor(
                            out=vt.rearrange("p j (h c) -> p j h c", h=2)[:, :, :, 0:64],
                            in0=ps.rearrange("p (j h c) -> p j h c", j=2, h=2),
                            in1=bv[:, g * 256:(g + 1) * 256]
                                .rearrange("p (j h c) -> p j h c", j=2, h=2),
                            op=ADD)
                        nc.vector.memset(vt[:, :, 64:65], 1.0)
                        nc.vector.memset(vt[:, :, 129:130], 1.0)
                        for j in range(2):
                            pr = 2 * g + j
                            nc.sync.dma_start(
                                out=cc_in[pr * PAIRSZ + KT_P:(pr + 1) * PAIRSZ]
                                    .rearrange("(s c) -> s c", c=130)[sb * 128:(sb + 1) * 128, :],
                                in_=vt[:, j, :])
                    for j in range(2):
                        pr = 2 * g + j
                        nc.gpsimd.collective_compute(
                            "AllGather", mybir.AluOpType.bypass, replica_groups=groups,
                            ins=[cc_in[pr * PAIRSZ:(pr + 1) * PAIRSZ]],
                            outs=[cc_outs[pr][:]],
                        )

                # q blocks (c-blocks 0..7) -> qT resident (overlaps AllGathers)
                for cb in range(8):
                    qk_block(cb, qT[:, cb, :])

            # ---- Phase 3: attention, one head-pair at a time ----
            with tc.tile_pool(name="kv", bufs=2) as kvpool, \
                 tc.tile_pool(name="at", bufs=4) as atpool, \
                 tc.tile_pool(name="misc", bufs=4) as mpool, \
                 tc.tile_pool(name="ps_s", bufs=4, space="PSUM") as spool, \
                 tc.tile_pool(name="ps_c", bufs=4, space="PSUM") as ctxpool:
                for p in range(NPAIR):
                    # kT_pair [128, 16, 128]: global kv-block order
                    ktp = kvpool.tile([128, 16, 128], bf16, tag="ktp", name=f"ktp_{p}")
                    for c in range(8):
                        r, off = chunk_owner(c)
                        nc.sync.dma_start(
                            out=ktp[:, 2 * c:2 * c + 2, :],
                            in_=kt_view(p, r)[:, off:off + CHUNK]
                                .rearrange("p (b k) -> p b k", b=2),
                        )
                    # v_ext [128, 16, 130]: slots in global kv order (ones baked in)
                    vxt = kvpool.tile([128, 16, 130], bf16, tag="vxt", name=f"vxt_{p}")
                    for c in range(8):
                        r, off = chunk_owner(c)
                        nc.sync.dma_start(
                            out=vxt[:, 2 * c:2 * c + 2, :],
                            in_=v_view(p, r)[off:off + CHUNK, :]
                                .rearrange("(o q) k -> q o k", q=128),
                        )

                    psc = [ctxpool.tile([65, 512], f32, tag="ctx", name=f"ctx_p{p}_{j}")
                           for j in range(2)]
                    for h in range(2):
                        nc.tensor.matmul(psc[h][:, :], zt[0:1, 0:65], zt[0:1, 0:512],
                                         start=True, stop=False, skip_group_check=True)

                    for t in range(NJP):
                        # one PSUM bank per HEAD per job-pair: the two
                        # tile_position-packed QK matmuls run concurrently in
                        # the PE array, so they must drain to different banks;
                        # same-bank writes (j0/j1 of one head) share a row
                        # group and serialize.
                        pss = [spool.tile([128, 512], f32, tag="s", name=f"s_{p}_{t}_{h}")
                               for h in range(2)]
                        for u in range(2):
                            i = 2 * t + u
                            kv = (15 - i) if i < 16 else (23 - i)
                            choff = CHUNK if i < 16 else 0
                            for h in range(2):
                                nc.tensor.matmul(
                                    pss[h][:, u * 256:(u + 1) * 256],
                                    ktp[h * 64:(h + 1) * 64, kv, :],
                                    qT[h * 64:(h + 1) * 64, p, choff:choff + CHUNK],
                                    start=True, stop=True, tile_position=(h * 64, 0),
                                )
                        # ats layout: [j0h0 | j1h0 | j0h1 | j1h1]
                        ats = atpool.tile([128, 1024], bf16, tag="at", name=f"at_{p}_{t}")
                        for h in range(2):
                            nc.scalar.activation(ats[:, h * 512:(h + 1) * 512],
                                                 pss[h][:], EXP, scale=0.125)
                        nc.vector.tensor_tensor(
                            out=ats[:], in0=ats[:], in1=masks[:, t, :], op=MULT)
                        for u in range(2):
                            i = 2 * t + u
                            kv = (15 - i) if i < 16 else (23 - i)
                            choff = CHUNK if i < 16 else 0
                            for h in range(2):
                                seg = 2 * h + u
                                nc.tensor.matmul(
                                    psc[h][:, choff:choff + CHUNK],
                                    vxt[:, kv, h * 65:h * 65 + 65],
                                    ats[:, seg * 256:(seg + 1) * 256],
                                    start=False, stop=(i == NJOB - 1),
                                    skip_group_check=True,
                                )

                    # evacuate unnormalized ctx + denominator row; den copies go
                    # on the (idle) Scalar engine so the pair-boundary evac
                    # chain runs DVE and ACT in parallel
                    for h in range(2):
                        nc.scalar.copy(
                            out=den_all[64:65, p, h, :], in_=psc[h][64:65, :])
                    for h in range(2):
                        nc.vector.tensor_copy(
                            out=ctxT[h * 64:(h + 1) * 64, p, :], in_=psc[h][0:64, :])

            # ---- Phase 3.5: deferred normalization ----
            with tc.tile_pool(name="nrm", bufs=2) as npool, \
                 tc.tile_pool(name="nrmp", bufs=2, space="PSUM") as npsp:
                den16 = npool.tile([16, SLOC], f32, tag="den16")
                nc.sync.dma_start(out=den_dram[:], in_=den_all[64:65, :, :, :])
                nc.sync.dma_start(out=den16[:], in_=den_dram[:])
                recf = npool.tile([16, SLOC], f32, tag="recf")
                rec = npool.tile([16, SLOC], bf16, tag="rec")
                with nc.allow_low_precision(reason="softmax denom"):
                    nc.vector.reciprocal(recf[:], den16[:])
                nc.vector.tensor_copy(out=rec[:], in_=recf[:])
                for p in range(NPAIR):
                    bcp = npsp.tile([128, 512], f32, tag="bc", name=f"bc_{p}")
                    nc.tensor.matmul(bcp[:], et[:, p * 128:(p + 1) * 128], rec[:],
                                     start=True, stop=True)
                    nc.vector.tensor_tensor(
                        out=ctxT[:, p, :], in0=ctxT[:, p, :], in1=bcp[:], op=MULT)

            # ---- Phase 4: output projection ----
            with tc.tile_pool(name="ph4w", bufs=8) as wpool, \
                 tc.tile_pool(name="ph4", bufs=3) as pool, \
                 tc.tile_pool(name="ph4p", bufs=2, space="PSUM") as psp:
                wo = []
                for cb in range(8):
                    w = wpool.tile([128, D], bf16, tag="wo", name=f"wo_{cb}")
                    nc.sync.dma_start(out=w[:], in_=wout_in[cb * 128:(cb + 1) * 128, :])
                    wo.append(w)
                for sb in range(4):
                    for nb in range(2):
                        ps = psp.tile([128, 512], f32, tag="y", name=f"psy_{sb}_{nb}")
                        for cb in range(8):
                            nc.tensor.matmul(ps[:], ctxT[:, cb, sb * 128:(sb + 1) * 128],
                                             wo[cb][:, nb * 512:(nb + 1) * 512],
                                             start=(cb == 0), stop=(cb == 7))
                        yt = pool.tile([128, 512], f32, tag="yt", name=f"yt_{sb}_{nb}")
                        nc.vector.tensor_copy(out=yt[:], in_=ps[:])
                        nc.sync.dma_start(
                            out=y_out[sb * 128:(sb + 1) * 128, nb * 512:(nb + 1) * 512],
                            in_=yt[:])

    nc.finalize()
    return nc


def _host_inputs(x, W_qkv, b_qkv, W_out):
    import ml_dtypes

    x = np.asarray(x, ml_dtypes.bfloat16)
    W_qkv = np.asarray(W_qkv, np.float32)
    b_qkv = np.asarray(b_qkv, np.float32)
    W_out = np.ascontiguousarray(np.asarray(W_out, ml_dtypes.bfloat16))

    # q/k panels: [p, cb, db, c] = W_qkv[db*128+p, cb*128+c] for cb in 0..15
    wqk = W_qkv[:, :2 * D].reshape(8, 128, 16, 128)          # [db, p, cb, c]
    wqk_p = np.ascontiguousarray(wqk.transpose(1, 2, 0, 3).astype(ml_dtypes.bfloat16))
    w_v = np.ascontiguousarray(W_qkv[:, 2 * D:].astype(ml_dtypes.bfloat16))

    bqk_t = np.ascontiguousarray(b_qkv[:2 * D].reshape(16, 128).T)  # [128, 16]
    bv_bc = np.ascontiguousarray(np.broadcast_to(b_qkv[2 * D:], (128, D)))

    in_maps = []
    for c in range(NCORES):
        b, l = divmod(c, 4)
        cA, cB = l, 7 - l
        x_local = np.ascontiguousarray(
            np.concatenate([x[b, cA * CHUNK:(cA + 1) * CHUNK],
                            x[b, cB * CHUNK:(cB + 1) * CHUNK]], axis=0))
        m2 = np.zeros((128, NJP, 4 * CHUNK), np.float32)
        pp = np.arange(128)[:, None]
        ff = np.arange(CHUNK)[None, :]
        for i in range(NJOB):
            if i < 16:
                kvb, r0 = 15 - i, cB * CHUNK
            else:
                kvb, r0 = 23 - i, cA * CHUNK
            blk = (128 * kvb + pp <= r0 + ff).astype(np.float32)  # [128, 256]
            t, u = divmod(i, 2)
            # ats segment order is [j0h0 | j1h0 | j0h1 | j1h1]
            m2[:, t, u * CHUNK:(u + 1) * CHUNK] = blk
            m2[:, t, (2 + u) * CHUNK:(3 + u) * CHUNK] = blk
        in_maps.append({
            "x_local": x_local,
            "w_qk_p": wqk_p,
            "w_v": w_v,
            "b_qk_t": bqk_t,
            "b_v_bc": bv_bc,
            "w_out": W_out,
            "masks2": m2.astype(ml_dtypes.bfloat16),
        })
    return in_maps


def _run(in_maps, trace=False):
    from concourse.bass_utils import run_bass_kernel_spmd

    if "nc" not in _CACHE:
        _CACHE["nc"] = _build_nc()
    return run_bass_kernel_spmd(_CACHE["nc"], in_maps, core_ids=list(range(NCORES)),
                                trace=trace)


def kernel(x, W_qkv, b_qkv, W_out):
    in_maps = _host_inputs(x, W_qkv, b_qkv, W_out)
    res = _run(in_maps)
    out = np.empty((B, S, D), np.float32)
    for c in range(NCORES):
        b, l = divmod(c, 4)
        y = res.results[c]["y"]
        out[b, l * CHUNK:(l + 1) * CHUNK] = y[0:CHUNK]
        out[b, (7 - l) * CHUNK:(8 - l) * CHUNK] = y[CHUNK:2 * CHUNK]
    return out



# revision 19
# speedup vs baseline: 1.1344x; 1.1344x over previous
"""Causal multi-head attention (B=2, S=2048, D=1024, H=16) on 8 TRN2 NeuronCores.

v3: head-parallel (tensor-parallel) sharding. Each core owns 2 heads for BOTH
batches: core c -> heads (2c, 2c+1). The full x is fed to every core from host
HBM (pre-transposed on host), so q/k/v for the core's heads are computed
locally and causal attention needs NO k/v collective at all (the v2 baseline
spent ~170us on 8 serialized AllGathers at ~45 GB/s mesh bandwidth).

After attention, ctx (normalized) is resharded from head-split to row-split
with ONE 8-way 1MB AllToAll (~13us), and the output projection runs locally
on each core's 512-row shard.

Attention structure per (batch, 256-row q-tile j): kv blocks 0..2j+1 processed
as job-pairs (jp = 2 kv blocks x 2 heads), QK packed 2-heads-per-PE-pass via
tile_position, exp on ACT in alternating [128,2048]/[128,1024] chunks (A/B
PSUM rings; ACT cost model is (N+352)/1.2GHz so bigger chunks amortize the
fixed overhead), softmax denominators via a ones-column in v, deferred
normalization via reciprocal + E-matrix broadcast matmul. Only the diagonal
job-pair needs a mask multiply; one inline [128,1024] mask tile serves every
q-tile/head/batch. b1's QKV projection matmuls are interleaved into b0's
ACT-paced attention chunks to fill PE bubbles.
"""

import numpy as np

B, S, D = 2, 2048, 1024
H = 16
HD = 64
NCORES = 8
QT = 256            # q-tile rows
NQT = S // QT       # 8 q-tiles per batch
KVB = 128           # kv block size
ROWS = 512          # output rows per core
SHARD = 128 * ROWS  # A2A shard elems (bf16)

_CACHE = {}


def _build_nc():
    import ml_dtypes
    import concourse.bass as bass
    import concourse.bacc as bacc
    import concourse.mybir as mybir
    import concourse.tile as tile

    f32 = mybir.dt.float32
    bf16 = mybir.dt.bfloat16
    MULT = mybir.AluOpType.mult
    ADD = mybir.AluOpType.add
    EXP = mybir.ActivationFunctionType.Exp

    nc = bacc.Bacc(num_devices=NCORES)

    xT_in = nc.dram_tensor("x_t", [128, B, 8, S], bf16, kind="ExternalInput")
    wq_in = nc.dram_tensor("w_q", [128, 8, 128], bf16, kind="ExternalInput")
    wk_in = nc.dram_tensor("w_k", [128, 8, 128], bf16, kind="ExternalInput")
    wv_in = nc.dram_tensor("w_v", [128, 8, 128], bf16, kind="ExternalInput")
    bq_in = nc.dram_tensor("b_q", [128, 1], f32, kind="ExternalInput")
    bk_in = nc.dram_tensor("b_k", [128, 1], f32, kind="ExternalInput")
    bv_in = nc.dram_tensor("b_v", [128, 128], f32, kind="ExternalInput")
    wo_in = nc.dram_tensor("w_o", [128, 8, D], bf16, kind="ExternalInput")
    y_out = nc.dram_tensor("y", [ROWS, D], f32, kind="ExternalOutput")

    cc_in = nc.dram_tensor("cc_in", [NCORES, SHARD], bf16)
    cc_out = nc.dram_tensor("cc_out", [NCORES, SHARD], bf16)

    import os
    KDBG = bool(os.environ.get("KDBG"))
    if KDBG:
        dbg_q = nc.dram_tensor("dbg_q", [128, B, S], bf16, kind="ExternalOutput")
        dbg_k = nc.dram_tensor("dbg_k", [128, B, S], bf16, kind="ExternalOutput")
        dbg_v = nc.dram_tensor("dbg_v", [128, B, 16, 130], bf16, kind="ExternalOutput")
        dbg_ctx = nc.dram_tensor("dbg_ctx", [128, B, S], bf16, kind="ExternalOutput")
        dbg_den = nc.dram_tensor("dbg_den", [65, B, S], f32, kind="ExternalOutput")
        dbg_ca = nc.dram_tensor("dbg_ca", [128, 8, ROWS], bf16, kind="ExternalOutput")
        dbg_mask = nc.dram_tensor("dbg_mask", [128, 1024], bf16, kind="ExternalOutput")
        dbg_ats = nc.dram_tensor("dbg_ats", [128, 1024], bf16, kind="ExternalOutput")
        dbg_sc = nc.dram_tensor("dbg_sc", [128, 1024], f32, kind="ExternalOutput")

    # diagonal-block mask: ats segment layout [u0h0 | u1h0 | u0h1 | u1h1],
    # seg (h,u): valid iff u*128 + p <= r
    m_np = np.zeros((128, 1024), np.float32)
    pp = np.arange(128)[:, None]
    rr = np.arange(256)[None, :]
    for h in range(2):
        for u in range(2):
            m_np[:, (2 * h + u) * 256:(2 * h + u + 1) * 256] = (u * 128 + pp <= rr)
    mask_h = nc.inline_tensor(m_np.astype(ml_dtypes.bfloat16), name="mask_c")
    # den rows live at partitions 0 (h0) and 64 (h1) — engine writes must start
    # at 32-aligned partitions. E broadcasts those rows to the head halves.
    e_np = np.zeros((65, 128), np.float32)
    e_np[0, 0:64] = 1.0
    e_np[64, 64:128] = 1.0
    e2_h = nc.inline_tensor(e_np.astype(ml_dtypes.bfloat16), name="e2_c")

    # chunk schedule: strict global A/B alternation (A=2 jps, B=1 jp) so the
    # two PSUM score rings pipeline; jp t covers kv blocks (2t, 2t+1)
    chunks = []
    parity = 0
    for b in range(B):
        for j in range(NQT):
            rem, t = j + 1, 0
            while rem:
                n = min(2, rem) if parity == 0 else 1
                chunks.append(dict(b=b, j=j, t0=t, n=n, kind="AB"[parity],
                                   last=(rem - n == 0)))
                t += n
                rem -= n
                parity ^= 1
    nb0 = sum(1 for c in chunks if c["b"] == 0)

    with tile.TileContext(nc) as tc:
        with tc.tile_pool(name="const", bufs=1) as cpool:
            xT = cpool.tile([128, B, 8, S], bf16)
            for b in range(B):
                for st in range(4):
                    nc.sync.dma_start(out=xT[:, b, :, st * 512:(st + 1) * 512],
                                      in_=xT_in[:, b, :, st * 512:(st + 1) * 512])
            wq = cpool.tile([128, 8, 128], bf16)
            nc.sync.dma_start(out=wq[:], in_=wq_in[:])
            wk = cpool.tile([128, 8, 128], bf16)
            nc.sync.dma_start(out=wk[:], in_=wk_in[:])
            wv = cpool.tile([128, 8, 128], bf16)
            nc.sync.dma_start(out=wv[:], in_=wv_in[:])
            bq = cpool.tile([128, 1], f32)
            nc.sync.dma_start(out=bq[:], in_=bq_in[:])
            bk = cpool.tile([128, 1], f32)
            nc.sync.dma_start(out=bk[:], in_=bk_in[:])
            bv = cpool.tile([128, 128], f32)
            nc.sync.dma_start(out=bv[:], in_=bv_in[:])
            wo = cpool.tile([128, 8, D], bf16)
            nc.sync.dma_start(out=wo[:], in_=wo_in[:])
            mask = cpool.tile([128, 1024], bf16)
            nc.sync.dma_start(out=mask[:], in_=mask_h[:])
            e2 = cpool.tile([65, 128], bf16)
            nc.sync.dma_start(out=e2[:], in_=e2_h[:])

            kT = cpool.tile([128, B, S], bf16)
            qT = cpool.tile([128, B, S], bf16)
            vx = cpool.tile([128, B, 16, 130], bf16)
            ctxT = cpool.tile([128, B, S], bf16)
            ctx_all = cpool.tile([128, 8, ROWS], bf16)
            den_b = cpool.tile([65, B, S], f32)
            recf = cpool.tile([65, S], f32)
            rec_b = cpool.tile([65, B, S], bf16)
            zt = cpool.tile([1, 512], bf16)
            nc.vector.memset(zt[:], 0.0)
            # partitions 1..63 of den_b are never written; keep them at 1.0 so
            # the full-tile reciprocal stays finite (NaN*0 would poison bcast)
            nc.vector.memset(den_b[:], 1.0)
            # ones columns for the softmax denominator rows (slots 64, 129)
            nc.vector.memset(vx[:, :, :, 64:65], 1.0)
            nc.vector.memset(vx[:, :, :, 129:130], 1.0)

            def emit_qk_proj(b, st, wpan, bias, dest, pool, tag):
                ps = pool.tile([128, 512], f32, tag=tag, name=f"ps_{tag}_{b}_{st}")
                for db in range(8):
                    nc.tensor.matmul(ps[:], wpan[:, db, :],
                                     xT[:, b, db, st * 512:(st + 1) * 512],
                                     start=(db == 0), stop=(db == 7))
                nc.vector.tensor_scalar_add(
                    dest[:, b, st * 512:(st + 1) * 512], ps[:], bias[:])

            def emit_v_proj(b, st, sb, pool, tag):
                off = st * 512 + sb * 128
                ps = pool.tile([128, 128], f32, tag=tag, name=f"psv_{b}_{st}_{sb}",
                               padded_shape=[128, 512])
                for db in range(8):
                    nc.tensor.matmul(ps[:], xT[:, b, db, off:off + 128],
                                     wv[:, db, :], start=(db == 0), stop=(db == 7))
                kvb = st * 4 + sb
                nc.vector.tensor_tensor(
                    out=vx[:, b, kvb].rearrange("p (h c) -> p h c", h=2)[:, :, 0:64],
                    in0=ps.rearrange("p (h c) -> p h c", h=2),
                    in1=bv.rearrange("p (h c) -> p h c", h=2),
                    op=ADD)

            # ---- Phase 1: batch-0 QKV projection ----
            with tc.tile_pool(name="p1ps", bufs=1, space="PSUM") as p1:
                for st in range(4):
                    emit_qk_proj(0, st, wk, bk, kT, p1, "qk0")
                    emit_qk_proj(0, st, wq, bq, qT, p1, "qk1")
                    for sb in range(4):
                        emit_v_proj(0, st, sb, p1, "v" + str(sb % 2))

            # ---- Phase 2: attention (b0 + b1-qkv interleaved, then b1) ----
            with tc.tile_pool(name="p2ps", bufs=1, space="PSUM") as p2, \
                 tc.tile_pool(name="ats", bufs=2) as apool:

                pieces = []
                for st in range(4):
                    pieces.append(lambda st=st: emit_qk_proj(1, st, wk, bk, kT, p2, "pk"))
                    pieces.append(lambda st=st: emit_qk_proj(1, st, wq, bq, qT, p2, "pk"))
                    for sb in range(4):
                        pieces.append(lambda st=st, sb=sb: emit_v_proj(1, st, sb, p2, "pk"))

                psc_cur = [None]

                def emit_chunk_qk(c, i):
                    b, j, t0, n = c["b"], c["j"], c["t0"], c["n"]
                    if c["kind"] == "A":
                        ps = p2.tile([128, 2048], f32, tag="scA", name=f"scA_{i}")
                    else:
                        ps = p2.tile([128, 1024], f32, tag="scB", name=f"scB_{i}")
                    ats = apool.tile([128, n * 1024], bf16, tag="at" + c["kind"],
                                     name=f"at_{i}")
                    for s in range(n):
                        t = t0 + s
                        for u in range(2):
                            kvb = 2 * t + u
                            for h in range(2):
                                nc.tensor.matmul(
                                    ps[:, s * 1024 + h * 512 + u * 256:
                                       s * 1024 + h * 512 + (u + 1) * 256],
                                    kT[h * 64:(h + 1) * 64, b, kvb * 128:(kvb + 1) * 128],
                                    qT[h * 64:(h + 1) * 64, b, j * QT:(j + 1) * QT],
                                    start=True, stop=True, tile_position=(h * 64, 0),
                                )
                    c["ps"] = ps
                    c["ats"] = ats

                def emit_chunk_tail(c):
                    b, j, t0, n = c["b"], c["j"], c["t0"], c["n"]
                    ps, ats = c["ps"], c["ats"]
                    nc.scalar.activation(ats[:], ps[:, 0:n * 1024], EXP)
                    if t0 + n - 1 == j:  # chunk contains the diagonal jp (t == j)
                        sd = j - t0
                        nc.vector.tensor_tensor(
                            out=ats[:, sd * 1024:(sd + 1) * 1024],
                            in0=ats[:, sd * 1024:(sd + 1) * 1024],
                            in1=mask[:], op=MULT)
                    if KDBG and b == 0 and j == 0:
                        nc.sync.dma_start(out=dbg_ats[:], in_=ats[:, 0:1024])
                        nc.sync.dma_start(out=dbg_mask[:], in_=mask[:])
                    if t0 == 0:
                        psc_cur[0] = p2.tile([65, 512], f32, tag="psc",
                                             name=f"psc_{b}_{j}")
                        # start=True resets has_written for the WHOLE 2KB bank,
                        # so a per-h start would wipe the other head's partial
                        # sums; zero-init the bank once instead.
                        nc.tensor.matmul(psc_cur[0][:, :], zt[0:1, 0:65],
                                         zt[0:1, 0:512], start=True, stop=False,
                                         skip_group_check=True)
                    psc = psc_cur[0]
                    for s in range(n):
                        t = t0 + s
                        for u in range(2):
                            kvb = 2 * t + u
                            for h in range(2):
                                nc.tensor.matmul(
                                    psc[:, h * 256:(h + 1) * 256],
                                    vx[:, b, kvb, h * 65:h * 65 + 65],
                                    ats[:, s * 1024 + h * 512 + u * 256:
                                        s * 1024 + h * 512 + (u + 1) * 256],
                                    start=False,
                                    stop=(t == j and u == 1),
                                    skip_group_check=True,
                                )
                    if c["last"]:
                        for h in range(2):
                            nc.vector.tensor_copy(
                                out=ctxT[h * 64:(h + 1) * 64, b, j * QT:(j + 1) * QT],
                                in_=psc[0:64, h * 256:(h + 1) * 256])
                            nc.vector.tensor_copy(
                                out=den_b[h * 64:h * 64 + 1, b, j * QT:(j + 1) * QT],
                                in_=psc[64:65, h * 256:(h + 1) * 256])

                def emit_norm(b):
                    with nc.allow_low_precision(reason="softmax denom"):
                        nc.vector.reciprocal(recf[:], den_b[:, b, :])
                    nc.vector.tensor_copy(out=rec_b[:, b, :], in_=recf[:])
                    for q4 in range(4):
                        bc = p2.tile([128, 512], f32, tag="pk", name=f"bc_{b}_{q4}")
                        nc.tensor.matmul(bc[:], e2[:],
                                         rec_b[0:65, b, q4 * 512:(q4 + 1) * 512],
                                         start=True, stop=True)
                        nc.vector.tensor_tensor(
                            out=ctxT[:, b, q4 * 512:(q4 + 1) * 512],
                            in0=ctxT[:, b, q4 * 512:(q4 + 1) * 512],
                            in1=bc[:], op=MULT)
                    for m in range(4):
                        dest = b * 4 + m
                        nc.sync.dma_start(
                            out=cc_in[dest, :].rearrange("(p s) -> p s", s=ROWS),
                            in_=ctxT[:, b, m * ROWS:(m + 1) * ROWS])

                prev = None
                pieces_left = list(pieces)
                b0_chunks_left = nb0
                norm0_at = None
                for i, c in enumerate(chunks):
                    emit_chunk_qk(c, i)
                    if c["b"] == 0 and pieces_left:
                        k = -(-len(pieces_left) // b0_chunks_left)
                        for _ in range(min(k, len(pieces_left))):
                            pieces_left.pop(0)()
                    if c["b"] == 0:
                        b0_chunks_left -= 1
                    if prev is not None:
                        emit_chunk_tail(prev)
                        if prev["b"] == 0 and c["b"] == 1:
                            norm0_at = i + 2
                    if norm0_at == i:
                        emit_norm(0)
                    prev = c
                emit_chunk_tail(prev)
                if norm0_at is not None and norm0_at >= len(chunks):
                    emit_norm(0)
                emit_norm(1)

            if KDBG:
                nc.sync.dma_start(out=dbg_q[:], in_=qT[:])
                nc.sync.dma_start(out=dbg_k[:], in_=kT[:])
                nc.sync.dma_start(out=dbg_v[:], in_=vx[:])
                nc.sync.dma_start(out=dbg_ctx[:], in_=ctxT[:])
                nc.sync.dma_start(out=dbg_den[:], in_=den_b[:])

            # ---- Phase 3: A2A reshard + output projection ----
            nc.gpsimd.collective_compute(
                "AllToAll", mybir.AluOpType.bypass,
                replica_groups=[list(range(NCORES))],
                ins=[cc_in[:]], outs=[cc_out[:]],
            )
            with tc.tile_pool(name="p3ps", bufs=2, space="PSUM") as p3, \
                 tc.tile_pool(name="p3sb", bufs=2) as p3sb:
                for l in range(8):
                    nc.sync.dma_start(
                        out=ctx_all[:, l, :],
                        in_=cc_out[l, :].rearrange("(p s) -> p s", s=ROWS))
                if KDBG:
                    nc.sync.dma_start(out=dbg_ca[:], in_=ctx_all[:])
                for rt in range(4):
                    for nh in range(2):
                        ps = p3.tile([128, 512], f32, tag="y", name=f"py_{rt}_{nh}")
                        for cb in range(8):
                            nc.tensor.matmul(
                                ps[:], ctx_all[:, cb, rt * 128:(rt + 1) * 128],
                                wo[:, cb, nh * 512:(nh + 1) * 512],
                                start=(cb == 0), stop=(cb == 7))
                        yt = p3sb.tile([128, 512], f32, tag="yt", name=f"yt_{rt}_{nh}")
                        nc.vector.tensor_copy(out=yt[:], in_=ps[:])
                        nc.sync.dma_start(
                            out=y_out[rt * 128:(rt + 1) * 128, nh * 512:(nh + 1) * 512],
                            in_=yt[:])

    nc.finalize()
    return nc


def _host_inputs(x, W_qkv, b_qkv, W_out):
    import ml_dtypes

    x = np.asarray(x, np.float32)
    W_qkv = np.asarray(W_qkv, np.float32)
    b_qkv = np.asarray(b_qkv, np.float32)
    W_out = np.asarray(W_out, np.float32)

    # xT[p, b, db, s] = x[b, s, db*128+p]
    xT = np.ascontiguousarray(
        x.transpose(2, 0, 1).reshape(8, 128, B, S).transpose(1, 2, 0, 3)
    ).astype(ml_dtypes.bfloat16)
    wo_p = np.ascontiguousarray(
        W_out.reshape(8, 128, D).transpose(1, 0, 2)).astype(ml_dtypes.bfloat16)

    SCALE = 1.0 / np.sqrt(HD)
    in_maps = []
    for c in range(NCORES):
        co = 128 * c
        wq = (W_qkv[:, co:co + 128] * SCALE).reshape(8, 128, 128).transpose(1, 0, 2)
        wk = W_qkv[:, D + co:D + co + 128].reshape(8, 128, 128).transpose(1, 0, 2)
        wv = W_qkv[:, 2 * D + co:2 * D + co + 128].reshape(8, 128, 128).transpose(1, 0, 2)
        in_maps.append({
            "x_t": xT,
            "w_q": np.ascontiguousarray(wq).astype(ml_dtypes.bfloat16),
            "w_k": np.ascontiguousarray(wk).astype(ml_dtypes.bfloat16),
            "w_v": np.ascontiguousarray(wv).astype(ml_dtypes.bfloat16),
            "b_q": np.ascontiguousarray((b_qkv[co:co + 128] * SCALE).reshape(128, 1)),
            "b_k": np.ascontiguousarray(b_qkv[D + co:D + co + 128].reshape(128, 1)),
            "b_v": np.ascontiguousarray(
                np.broadcast_to(b_qkv[2 * D + co:2 * D + co + 128], (128, 128))),
            "w_o": wo_p,
        })
    return in_maps


def _run(in_maps, trace=False):
    from concourse.bass_utils import run_bass_kernel_spmd

    if "nc" not in _CACHE:
        _CACHE["nc"] = _build_nc()
    return run_bass_kernel_spmd(_CACHE["nc"], in_maps, core_ids=list(range(NCORES)),
                                trace=trace)


def _gather(res):
    out = np.empty((B, S, D), np.float32)
    for c in range(NCORES):
        b, m = divmod(c, 4)
        out[b, m * ROWS:(m + 1) * ROWS, :] = res.results[c]["y"]
    return out


def kernel(x, W_qkv, b_qkv, W_out):
    in_maps = _host_inputs(x, W_qkv, b_qkv, W_out)
    res = _run(in_maps)
    return _gather(res)


# revision 29
# speedup vs baseline: 1.3112x; 1.1559x over previous
"""Causal multi-head attention (B=2, S=2048, D=1024, H=16) on 8 TRN2 NeuronCores.

v3: head-parallel (tensor-parallel) sharding. Each core owns 2 heads for BOTH
batches: core c -> heads (2c, 2c+1). The full x is fed to every core from host
HBM (pre-transposed on host), so q/k/v for the core's heads are computed
locally and causal attention needs NO k/v collective at all (the v2 baseline
spent ~170us on 8 serialized AllGathers at ~45 GB/s mesh bandwidth).

After attention, ctx (normalized) is resharded from head-split to row-split
with ONE 8-way 1MB AllToAll (~13us), and the output projection runs locally
on each core's 512-row shard.

Attention structure per (batch, 256-row q-tile j): kv blocks 0..2j+1 processed
as job-pairs (jp = 2 kv blocks x 2 heads), QK packed 2-heads-per-PE-pass via
tile_position, exp on ACT in alternating [128,2048]/[128,1024] chunks (A/B
PSUM rings; ACT cost model is (N+352)/1.2GHz so bigger chunks amortize the
fixed overhead), softmax denominators via a ones-column in v, deferred
normalization via reciprocal + E-matrix broadcast matmul. Only the diagonal
job-pair needs a mask multiply; one inline [128,1024] mask tile serves every
q-tile/head/batch. b1's QKV projection matmuls are interleaved into b0's
ACT-paced attention chunks to fill PE bubbles.
"""

import numpy as np

B, S, D = 2, 2048, 1024
H = 16
HD = 64
NCORES = 8
QT = 256            # q-tile rows
NQT = S // QT       # 8 q-tiles per batch
KVB = 128           # kv block size
ROWS = 512          # output rows per core
SHARD = 128 * ROWS  # A2A shard elems (bf16)

_CACHE = {}


def _build_nc():
    import ml_dtypes
    import concourse.bass as bass
    import concourse.bacc as bacc
    import concourse.mybir as mybir
    import concourse.tile as tile

    f32 = mybir.dt.float32
    bf16 = mybir.dt.bfloat16
    MULT = mybir.AluOpType.mult
    ADD = mybir.AluOpType.add
    EXP = mybir.ActivationFunctionType.Exp

    nc = bacc.Bacc(num_devices=NCORES)

    xT_in = nc.dram_tensor("x_t", [128, B, 8, S], bf16, kind="ExternalInput")
    wq_in = nc.dram_tensor("w_q", [128, 8, 128], bf16, kind="ExternalInput")
    wk_in = nc.dram_tensor("w_k", [128, 8, 128], bf16, kind="ExternalInput")
    wv_in = nc.dram_tensor("w_v", [128, 8, 128], bf16, kind="ExternalInput")
    bq_in = nc.dram_tensor("b_q", [128, 1], f32, kind="ExternalInput")
    bk_in = nc.dram_tensor("b_k", [128, 1], f32, kind="ExternalInput")
    bv_in = nc.dram_tensor("b_v", [128, 1], f32, kind="ExternalInput")
    wo_in = nc.dram_tensor("w_o", [128, 8, D], bf16, kind="ExternalInput")
    y_out = nc.dram_tensor("y", [ROWS, D], f32, kind="ExternalOutput")

    cc_in = nc.dram_tensor("cc_in", [NCORES, SHARD], bf16)
    cc_out = nc.dram_tensor("cc_out", [NCORES, SHARD], bf16)

    import os
    KDBG = bool(os.environ.get("KDBG"))
    if KDBG:
        dbg_q = nc.dram_tensor("dbg_q", [128, B, S], bf16, kind="ExternalOutput")
        dbg_k = nc.dram_tensor("dbg_k", [128, B, S], bf16, kind="ExternalOutput")
        dbg_v = nc.dram_tensor("dbg_v", [128, B, 16, 130], bf16, kind="ExternalOutput")
        dbg_ctx = nc.dram_tensor("dbg_ctx", [128, B, S], bf16, kind="ExternalOutput")
        dbg_den = nc.dram_tensor("dbg_den", [65, B, S], f32, kind="ExternalOutput")
        dbg_ca = nc.dram_tensor("dbg_ca", [128, 8, ROWS], bf16, kind="ExternalOutput")
        dbg_mask = nc.dram_tensor("dbg_mask", [128, 1024], bf16, kind="ExternalOutput")
        dbg_ats = nc.dram_tensor("dbg_ats", [128, 1024], bf16, kind="ExternalOutput")
        dbg_sc = nc.dram_tensor("dbg_sc", [128, 1024], f32, kind="ExternalOutput")

    # diagonal-block mask: ats segment layout [u0h0 | u1h0 | u0h1 | u1h1],
    # seg (h,u): valid iff u*128 + p <= r
    m_np = np.zeros((128, 1024), np.float32)
    pp = np.arange(128)[:, None]
    rr = np.arange(256)[None, :]
    for h in range(2):
        for u in range(2):
            m_np[:, (2 * h + u) * 256:(2 * h + u + 1) * 256] = (u * 128 + pp <= rr)
    mask_h = nc.inline_tensor(m_np.astype(ml_dtypes.bfloat16), name="mask_c")
    # den rows live at partitions 0 (h0) and 64 (h1) — engine writes must start
    # at 32-aligned partitions. E broadcasts those rows to the head halves.
    e_np = np.zeros((65, 128), np.float32)
    e_np[0, 0:64] = 1.0
    e_np[64, 64:128] = 1.0
    e2_h = nc.inline_tensor(e_np.astype(ml_dtypes.bfloat16), name="e2_c")

    # chunk schedule: strict global A/B alternation (A=2 jps, B=1 jp) so the
    # two PSUM score rings pipeline; jp t covers kv blocks (2t, 2t+1)
    chunks = []
    parity = 0
    for b in range(B):
        for j in range(NQT):
            rem, t = j + 1, 0
            while rem:
                n = min(2, rem) if parity == 0 else 1
                chunks.append(dict(b=b, j=j, t0=t, n=n, kind="AB"[parity],
                                   last=(rem - n == 0)))
                t += n
                rem -= n
                parity ^= 1
    nb0 = sum(1 for c in chunks if c["b"] == 0)

    with tile.TileContext(nc) as tc:
        with tc.tile_pool(name="const", bufs=1) as cpool, \
             tc.tile_pool(name="vsp", bufs=2) as vspool:
            # small weight/bias/mask DMAs first: the first qkv matmul needs wk,
            # not the 8MB of xT
            wq = cpool.tile([128, 8, 128], bf16)
            nc.sync.dma_start(out=wq[:], in_=wq_in[:])
            wk = cpool.tile([128, 8, 128], bf16)
            nc.sync.dma_start(out=wk[:], in_=wk_in[:])
            wv = cpool.tile([128, 8, 128], bf16)
            nc.sync.dma_start(out=wv[:], in_=wv_in[:])
            bq = cpool.tile([128, 1], f32)
            nc.sync.dma_start(out=bq[:], in_=bq_in[:])
            bk = cpool.tile([128, 1], f32)
            nc.sync.dma_start(out=bk[:], in_=bk_in[:])
            bv = cpool.tile([128, 1], f32)
            nc.sync.dma_start(out=bv[:], in_=bv_in[:])
            mask = cpool.tile([128, 1024], bf16)
            nc.sync.dma_start(out=mask[:], in_=mask_h[:])
            e2 = cpool.tile([65, 128], bf16)
            nc.sync.dma_start(out=e2[:], in_=e2_h[:])
            xT = cpool.tile([128, B, 8, S], bf16)
            for b in range(B):
                for st in range(4):
                    nc.sync.dma_start(out=xT[:, b, :, st * 512:(st + 1) * 512],
                                      in_=xT_in[:, b, :, st * 512:(st + 1) * 512])
            wo = cpool.tile([128, 8, D], bf16)
            nc.sync.dma_start(out=wo[:], in_=wo_in[:])

            kT = cpool.tile([128, B, S], bf16)
            qT = cpool.tile([128, B, S], bf16)
            vT = cpool.tile([128, B, S], bf16)
            vx = cpool.tile([128, B, 16, 130], bf16)
            ctxT = cpool.tile([128, B, S], bf16)
            ctx_all = cpool.tile([128, 8, ROWS], bf16)
            den_b = cpool.tile([65, B, S], f32)
            recf = cpool.tile([65, S], f32)
            rec_b = cpool.tile([65, B, S], bf16)
            zt = cpool.tile([1, 512], bf16)
            nc.vector.memset(zt[:], 0.0)
            # partitions 1..63 of den_b are never written; keep them at 1.0 so
            # the full-tile reciprocal stays finite (NaN*0 would poison bcast)
            nc.vector.memset(den_b[:], 1.0)
            # ones columns for the softmax denominator rows (slots 64, 129)
            nc.vector.memset(vx[:, :, :, 64:65], 1.0)
            nc.vector.memset(vx[:, :, :, 129:130], 1.0)

            def emit_qk_proj(b, st, wpan, bias, dest, pool, tag):
                ps = pool.tile([128, 512], f32, tag=tag, name=f"ps_{tag}_{b}_{st}")
                for db in range(8):
                    nc.tensor.matmul(ps[:], wpan[:, db, :],
                                     xT[:, b, db, st * 512:(st + 1) * 512],
                                     start=(db == 0), stop=(db == 7))
                nc.vector.tensor_scalar_add(
                    dest[:, b, st * 512:(st + 1) * 512], ps[:], bias[:])

            def emit_v_proj(b, st, pool, tag):
                # channel-major v (bias is per-partition), then DMA-transpose
                # 128x128 tiles and DVE-copy into the kv-major v_ext layout
                emit_qk_proj(b, st, wv, bv, vT, pool, tag)
                for sb in range(4):
                    kvb = st * 4 + sb
                    vs = vspool.tile([128, 128], bf16, tag="vs", name=f"vs_{b}_{kvb}")
                    nc.sync.dma_start_transpose(
                        out=vs[:], in_=vT[:, b, kvb * 128:(kvb + 1) * 128])
                    nc.vector.tensor_copy(
                        out=vx[:, b, kvb].rearrange("p (h c) -> p h c", h=2)[:, :, 0:64],
                        in_=vs.rearrange("p (h c) -> p h c", h=2))

            # ---- Phase 1: batch-0 QKV projection ----
            with tc.tile_pool(name="p1ps", bufs=1, space="PSUM") as p1:
                for st in range(4):
                    emit_qk_proj(0, st, wk, bk, kT, p1, "qk0")
                    emit_v_proj(0, st, p1, "qk1")
                    emit_qk_proj(0, st, wq, bq, qT, p1, "qk0")

            # ---- Phase 2: attention (b0 + b1-qkv interleaved, then b1) ----
            with tc.tile_pool(name="p2ps", bufs=1, space="PSUM") as p2, \
                 tc.tile_pool(name="ats", bufs=2) as apool:

                # b1 qkv pieces fill PE bubbles of the ACT-paced attention.
                # Piece for stage st must be emitted before chunk (b1, j=2*st)
                # (q-tile 2s reads q/k/v of stage s).
                pieces = []
                for st in range(4):
                    pieces.append((st, lambda st=st: emit_qk_proj(1, st, wk, bk, kT, p2, "pk")))
                    pieces.append((st, lambda st=st: emit_v_proj(1, st, p2, "pk")))
                    pieces.append((st, lambda st=st: emit_qk_proj(1, st, wq, bq, qT, p2, "pk")))

                psc_cur = [None]

                def emit_chunk_qk(c, i):
                    b, j, t0, n = c["b"], c["j"], c["t0"], c["n"]
                    if c["kind"] == "A":
                        ps = p2.tile([128, 2048], f32, tag="scA", name=f"scA_{i}")
                    else:
                        ps = p2.tile([128, 1024], f32, tag="scB", name=f"scB_{i}")
                    ats = apool.tile([128, n * 1024], bf16, tag="at" + c["kind"],
                                     name=f"at_{i}")
                    for s in range(n):
                        t = t0 + s
                        for u in range(2):
                            kvb = 2 * t + u
                            for h in range(2):
                                nc.tensor.matmul(
                                    ps[:, s * 1024 + h * 512 + u * 256:
                                       s * 1024 + h * 512 + (u + 1) * 256],
                                    kT[h * 64:(h + 1) * 64, b, kvb * 128:(kvb + 1) * 128],
                                    qT[h * 64:(h + 1) * 64, b, j * QT:(j + 1) * QT],
                                    start=True, stop=True, tile_position=(h * 64, 0),
                                )
                    c["ps"] = ps
                    c["ats"] = ats

                def emit_chunk_tail(c):
                    b, j, t0, n = c["b"], c["j"], c["t0"], c["n"]
                    ps, ats = c["ps"], c["ats"]
                    nc.scalar.activation(ats[:], ps[:, 0:n * 1024], EXP)
                    if t0 + n - 1 == j:  # chunk contains the diagonal jp (t == j)
                        sd = j - t0
                        nc.vector.tensor_tensor(
                            out=ats[:, sd * 1024:(sd + 1) * 1024],
                            in0=ats[:, sd * 1024:(sd + 1) * 1024],
                            in1=mask[:], op=MULT)
                    if KDBG and b == 0 and j == 0:
                        nc.sync.dma_start(out=dbg_ats[:], in_=ats[:, 0:1024])
                        nc.sync.dma_start(out=dbg_mask[:], in_=mask[:])
                    if t0 == 0:
                        psc_cur[0] = p2.tile([65, 512], f32, tag="psc",
                                             name=f"psc_{b}_{j}")
                        # start=True resets has_written for the WHOLE 2KB bank,
                        # so a per-h start would wipe the other head's partial
                        # sums; zero-init the bank once instead.
                        nc.tensor.matmul(psc_cur[0][:, :], zt[0:1, 0:65],
                                         zt[0:1, 0:512], start=True, stop=False,
                                         skip_group_check=True)
                    psc = psc_cur[0]
                    for s in range(n):
                        t = t0 + s
                        for u in range(2):
                            kvb = 2 * t + u
                            for h in range(2):
                                nc.tensor.matmul(
                                    psc[:, h * 256:(h + 1) * 256],
                                    vx[:, b, kvb, h * 65:h * 65 + 65],
                                    ats[:, s * 1024 + h * 512 + u * 256:
                                        s * 1024 + h * 512 + (u + 1) * 256],
                                    start=False,
                                    stop=(t == j and u == 1),
                                    skip_group_check=True,
                                )
                    if c["last"]:
                        for h in range(2):
                            nc.vector.tensor_copy(
                                out=ctxT[h * 64:(h + 1) * 64, b, j * QT:(j + 1) * QT],
                                in_=psc[0:64, h * 256:(h + 1) * 256])
                            nc.vector.tensor_copy(
                                out=den_b[h * 64:h * 64 + 1, b, j * QT:(j + 1) * QT],
                                in_=psc[64:65, h * 256:(h + 1) * 256])

                def emit_norm(b):
                    # ~18 correct bits, 5x faster than reciprocal(); den>0 always
                    nc.vector.reciprocal_approx_fast(recf[:], den_b[:, b, :])
                    nc.vector.tensor_copy(out=rec_b[:, b, :], in_=recf[:])
                    for q4 in range(4):
                        bc = p2.tile([128, 512], f32, tag="pk", name=f"bc_{b}_{q4}")
                        nc.tensor.matmul(bc[:], e2[:],
                                         rec_b[0:65, b, q4 * 512:(q4 + 1) * 512],
                                         start=True, stop=True)
                        nc.vector.tensor_tensor(
                            out=ctxT[:, b, q4 * 512:(q4 + 1) * 512],
                            in0=ctxT[:, b, q4 * 512:(q4 + 1) * 512],
                            in1=bc[:], op=MULT)
                    for m in range(4):
                        dest = b * 4 + m
                        nc.sync.dma_start(
                            out=cc_in[dest, :].rearrange("(p s) -> p s", s=ROWS),
                            in_=ctxT[:, b, m * ROWS:(m + 1) * ROWS])

                # piece-slot plan: spread the 12 b1-qkv pieces over b0's chunks
                # and b1's early chunks, forced-flushing by deadline
                prev = None
                pieces_left = list(pieces)
                norm0_at = None
                for i, c in enumerate(chunks):
                    if c["b"] == 1 and c["t0"] == 0:
                        while pieces_left and pieces_left[0][0] <= c["j"] // 2:
                            pieces_left.pop(0)[1]()
                    emit_chunk_qk(c, i)
                    if pieces_left and i % 2 == 0:
                        pieces_left.pop(0)[1]()
                    if prev is not None:
                        emit_chunk_tail(prev)
                        if prev["b"] == 0 and c["b"] == 1:
                            norm0_at = i + 2
                    if norm0_at == i:
                        emit_norm(0)
                    prev = c
                emit_chunk_tail(prev)
                if norm0_at is not None and norm0_at >= len(chunks):
                    emit_norm(0)
                emit_norm(1)

            if KDBG:
                nc.sync.dma_start(out=dbg_q[:], in_=qT[:])
                nc.sync.dma_start(out=dbg_k[:], in_=kT[:])
                nc.sync.dma_start(out=dbg_v[:], in_=vx[:])
                nc.sync.dma_start(out=dbg_ctx[:], in_=ctxT[:])
                nc.sync.dma_start(out=dbg_den[:], in_=den_b[:])

            # ---- Phase 3: A2A reshard + output projection ----
            nc.gpsimd.collective_compute(
                "AllToAll", mybir.AluOpType.bypass,
                replica_groups=[list(range(NCORES))],
                ins=[cc_in[:]], outs=[cc_out[:]],
            )
            with tc.tile_pool(name="p3ps", bufs=2, space="PSUM") as p3, \
                 tc.tile_pool(name="p3sb", bufs=2) as p3sb:
                for l in range(8):
                    nc.sync.dma_start(
                        out=ctx_all[:, l, :],
                        in_=cc_out[l, :].rearrange("(p s) -> p s", s=ROWS))
                if KDBG:
                    nc.sync.dma_start(out=dbg_ca[:], in_=ctx_all[:])
                for rt in range(4):
                    for nh in range(2):
                        ps = p3.tile([128, 512], f32, tag="y", name=f"py_{rt}_{nh}")
                        for cb in range(8):
                            nc.tensor.matmul(
                                ps[:], ctx_all[:, cb, rt * 128:(rt + 1) * 128],
                                wo[:, cb, nh * 512:(nh + 1) * 512],
                                start=(cb == 0), stop=(cb == 7))
                        yt = p3sb.tile([128, 512], f32, tag="yt", name=f"yt_{rt}_{nh}")
                        nc.vector.tensor_copy(out=yt[:], in_=ps[:])
                        nc.sync.dma_start(
                            out=y_out[rt * 128:(rt + 1) * 128, nh * 512:(nh + 1) * 512],
                            in_=yt[:])

    nc.finalize()
    return nc


def _host_inputs(x, W_qkv, b_qkv, W_out):
    import ml_dtypes

    x = np.asarray(x, np.float32)
    W_qkv = np.asarray(W_qkv, np.float32)
    b_qkv = np.asarray(b_qkv, np.float32)
    W_out = np.asarray(W_out, np.float32)

    # xT[p, b, db, s] = x[b, s, db*128+p]
    xT = np.ascontiguousarray(
        x.transpose(2, 0, 1).reshape(8, 128, B, S).transpose(1, 2, 0, 3)
    ).astype(ml_dtypes.bfloat16)
    wo_p = np.ascontiguousarray(
        W_out.reshape(8, 128, D).transpose(1, 0, 2)).astype(ml_dtypes.bfloat16)

    SCALE = 1.0 / np.sqrt(HD)
    in_maps = []
    for c in range(NCORES):
        co = 128 * c
        wq = (W_qkv[:, co:co + 128] * SCALE).reshape(8, 128, 128).transpose(1, 0, 2)
        wk = W_qkv[:, D + co:D + co + 128].reshape(8, 128, 128).transpose(1, 0, 2)
        wv = W_qkv[:, 2 * D + co:2 * D + co + 128].reshape(8, 128, 128).transpose(1, 0, 2)
        in_maps.append({
            "x_t": xT,
            "w_q": np.ascontiguousarray(wq).astype(ml_dtypes.bfloat16),
            "w_k": np.ascontiguousarray(wk).astype(ml_dtypes.bfloat16),
            "w_v": np.ascontiguousarray(wv).astype(ml_dtypes.bfloat16),
            "b_q": np.ascontiguousarray((b_qkv[co:co + 128] * SCALE).reshape(128, 1)),
            "b_k": np.ascontiguousarray(b_qkv[D + co:D + co + 128].reshape(128, 1)),
            "b_v": np.ascontiguousarray(b_qkv[2 * D + co:2 * D + co + 128].reshape(128, 1)),
            "w_o": wo_p,
        })
    return in_maps


def _run(in_maps, trace=False):
    from concourse.bass_utils import run_bass_kernel_spmd

    if "nc" not in _CACHE:
        _CACHE["nc"] = _build_nc()
    return run_bass_kernel_spmd(_CACHE["nc"], in_maps, core_ids=list(range(NCORES)),
                                trace=trace)


def _gather(res):
    out = np.empty((B, S, D), np.float32)
    for c in range(NCORES):
        b, m = divmod(c, 4)
        out[b, m * ROWS:(m + 1) * ROWS, :] = res.results[c]["y"]
    return out


def kernel(x, W_qkv, b_qkv, W_out):
    in_maps = _host_inputs(x, W_qkv, b_qkv, W_out)
    res = _run(in_maps)
    return _gather(res)
